# revision 39
# baseline (speedup 1.0000x reference)
"""Trainium2 Bass kernel for nn_Attention2d.

Computation: GroupNorm(32 groups) -> 1x1 qkv conv -> 4-head attention over
H*W=4096 positions -> 1x1 proj conv -> residual add.

Sharding: one (batch, head) pair per NeuronCore (B=2 x NH=4 = 8 cores).
Each core:
  - GroupNorm stats of its batch slice; the affine normalization is folded
    into the qkv weights (W' = W*s per input channel) and effective biases
    (b' = W@t + b), so the x-sized tensor is only cast to fp16 once
  - its head's q/k (with effective bias) and v^T (bias exported to host)
  - S^T = k^T q in [keys-on-partitions, queries-on-free] layout
    (no max-subtraction: |S/8| <~ 6 so exp is safe in fp32)
  - P^T = exp(S^T/8); PV via matmul with lhsT = [v^T | ones]  -> the ones
    column yields the softmax denominators for free (row 64 of the output)
  - proj partial = Wp[:, head]^T @ PV_raw  (un-normalized)
Host: out[b] = x[b] + proj_bias + sum_h (partial_h/denom_h + Wp_h @ bve_h)
(the softmax normalization and the constant v-bias commute through proj).

PE-array packing: the S matmuls contract over only hd=64 partitions, so q and
k are duplicated onto partitions 64..127 (by col-packed qkv matmuls that cost
no extra PE time) and each S^T tile is computed as two concurrent matmuls on
row-groups (0,0) and (64,0).

Matmul dtypes: qkv/attention/proj matmuls use float16 operands (1 cycle/row,
fast weight loads); GroupNorm matmuls, softmax denominators and all
reductions stay fp32. GroupNorm's rsqrt runs on the DVE (bit-trick seed +
Newton) so the ScalarE keeps a single Exp table set for the whole kernel.
"""

import numpy as np

B, C, H, W = 2, 256, 64, 64
HW = H * W           # 4096
GROUPS = 32
NH = 4
HD = C // NH         # 64
EPS = 1e-5
P = 128
IB = 1024            # query block (PSUM-sized)
NIB = HW // IB       # 4
NJC = HW // P        # 32 key chunks
NCORES = B * NH

# "f32": exact fp32 everywhere (slow). "f32r": float32r operands
# (TF32-like rounding, ~3 cycles/row on HW). "f16": float16 operands.
MM_MODE = "f16"

# aux column layout: 0 bq2, 1 bk2, 2 bv (rows 0:64), 3+po gnw, 5+po gnb,
# 7+32*po gmat
NAUX = 7 + 2 * GROUPS

_module_cache = {}


def _build_module(mm=MM_MODE):
    import concourse.bacc as bacc
    import concourse.tile as tile
    import concourse.mybir as mybir

    dt = mybir.dt
    f32 = dt.float32
    AF = mybir.ActivationFunctionType
    OP = mybir.AluOpType
    if mm == "f32":
        adt = f32
    elif mm == "f32r":
        adt = dt.float32r
    elif mm == "f16":
        adt = dt.float16
    else:
        raise ValueError(mm)

    nc = bacc.Bacc(trn_type="TRN2", target_bir_lowering=False, debug=False)

    # ---- DRAM I/O (per-core tensors; host prepares layouts) ----
    # channel layout everywhere: c = po*128 + pi  ->  [pi, po, ...]
    # x arrives already cast to the attention dtype (host-side cast); the
    # GroupNorm statistics absorb the rounding (it averages out over 32k
    # elements per group).
    x_d = nc.dram_tensor("x", [P, 2, HW], adt, kind="ExternalInput").ap()
    # packed raw qkv weight slices: [wq | wk | wv] along the last axis, fp32
    wqkv_d = nc.dram_tensor("wqkv", [P, 2, 3 * HD], f32, kind="ExternalInput").ap()
    wp_d = nc.dram_tensor("wp", [HD, C], adt if mm == "f16" else f32,
                          kind="ExternalInput").ap()
    aux_d = nc.dram_tensor("aux", [P, NAUX], f32, kind="ExternalInput").ap()
    gbc_d = nc.dram_tensor("gbc", [GROUPS, 2, P], f32, kind="ExternalInput").ap()
    out_d = nc.dram_tensor("out", [P, 2, HW], f32, kind="ExternalOutput").ap()
    den_d = nc.dram_tensor("den", [NIB, IB], f32, kind="ExternalOutput").ap()
    bve_d = nc.dram_tensor("bve", [HD, 1], f32, kind="ExternalOutput").ap()

    with tile.TileContext(nc) as tc:
        with (
            tc.tile_pool(name="const", bufs=1) as const,
            tc.tile_pool(name="big", bufs=1) as big,
            tc.tile_pool(name="tmp", bufs=3) as tmp,
            tc.tile_pool(name="pt", bufs=4) as ptp,
            tc.tile_pool(name="oh", bufs=2) as ohp,
            tc.tile_pool(name="ostage", bufs=3) as ostage,
            tc.tile_pool(name="ps_st", bufs=2, space="PSUM") as ps_st,
            tc.tile_pool(name="ps_pv", bufs=1, space="PSUM") as ps_pv,
            tc.tile_pool(name="ps_sm", bufs=2, space="PSUM") as ps_sm,
        ):
            eps_sb = const.tile([GROUPS, 1], f32)
            nc.vector.memset(eps_sb, EPS)
            ones_sb = const.tile([P, 1], f32)
            nc.vector.memset(ones_sb, 1.0)
            # Touch Exp immediately so walrus's single ACT table load runs
            # during the DMA-in phase.
            warm_sb = tmp.tile([GROUPS, 1], f32, tag="warm")
            nc.scalar.activation(out=warm_sb, in_=eps_sb, func=AF.Exp, scale=1.0)

            # ---- x first (chunked so stats start early); per-channel
            # moment accumulation split across DVE (po=0, bn_stats) and
            # ScalarE (po=1, Identity/Square with accum_out) ----
            x16 = big.tile([P, 2, HW], adt)
            stats0 = tmp.tile([P, 8, 6], f32, tag="bnstats0", name="stats0")
            stats1 = tmp.tile([P, 4, 6], f32, tag="bnstats1", name="stats1")
            psums1 = tmp.tile([P, 2], f32, tag="psums1", name="psums1")
            scratch = tmp.tile([P, HW // 2], adt, tag="scratch", name="scratch")
            HB = HW // 2
            for c in range(2):
                cs = slice(c * HB, (c + 1) * HB)
                # po=1 via the ScalarE HWDGE queue (parallel issue with po=0)
                nc.scalar.dma_start(x16[:, 1, cs], x_d[:, 1, cs])
                nc.sync.dma_start(x16[:, 0, cs], x_d[:, 0, cs])
                if c == 0:
                    # first po=1 half: two ScalarE accumulation passes
                    nc.scalar.activation(out=scratch, in_=x16[:, 1, cs],
                                         func=AF.Identity,
                                         accum_out=psums1[:, 0:1])
                    nc.scalar.activation(out=scratch, in_=x16[:, 1, cs],
                                         func=AF.Square,
                                         accum_out=psums1[:, 1:2])
                else:
                    # second po=1 half: DVE bn_stats (ScalarE is the long pole)
                    for s in range(4):
                        nc.vector.bn_stats(
                            out=stats1[:, s, :],
                            in_=x16[:, 1, HB + s * 512:HB + (s + 1) * 512],
                        )
                for s in range(4 * c, 4 * c + 4):
                    nc.vector.bn_stats(
                        out=stats0[:, s, :],
                        in_=x16[:, 0, s * 512:(s + 1) * 512],
                    )
            # weights ride the ScalarE HWDGE queue so the Sync queue stays
            # dedicated to x
            wqkv_sb = const.tile([P, 2, 3 * HD], f32)
            nc.scalar.dma_start(wqkv_sb, wqkv_d)
            aux_sb = const.tile([P, NAUX], f32)
            nc.scalar.dma_start(aux_sb, aux_d)
            gbc_sb = const.tile([GROUPS, 2, P], f32)
            nc.scalar.dma_start(gbc_sb, gbc_d)
            wp_sb = const.tile([HD, C], adt if mm == "f16" else f32)
            nc.scalar.dma_start(wp_sb, wp_d)

            # per-channel [mean_c, E[x^2]_c] for both po halves.
            # po=0: straight bn_aggr. po=1: combine ScalarE raw sums (first
            # half) with bn stats (second half):
            #   mean = s0/4096 + mean_c1/2;  Ex2 = q0/4096 + (var+mean^2)_c1/2
            mv0 = const.tile([P, 2], f32, tag="mv0")
            nc.vector.bn_aggr(out=mv0, in_=stats0)
            msq = tmp.tile([P, 1], f32, tag="msq")
            nc.vector.tensor_mul(msq, mv0[:, 0:1], mv0[:, 0:1])
            nc.vector.tensor_add(mv0[:, 1:2], mv0[:, 1:2], msq)
            mvq = tmp.tile([P, 2], f32, tag="mvq", name="mvq")
            nc.vector.bn_aggr(out=mvq, in_=stats1)
            msq1 = tmp.tile([P, 1], f32, tag="msq1", name="msq1")
            nc.vector.tensor_mul(msq1, mvq[:, 0:1], mvq[:, 0:1])
            nc.vector.tensor_add(mvq[:, 1:2], mvq[:, 1:2], msq1)  # Ex2_c1
            mv1 = const.tile([P, 2], f32, tag="mv1")
            for col in range(2):
                half_sb = tmp.tile([P, 1], f32, tag="halfsb", name="half_sb")
                nc.vector.tensor_scalar_mul(half_sb, mvq[:, col:col + 1], 0.5)
                nc.vector.scalar_tensor_tensor(
                    out=mv1[:, col:col + 1], in0=psums1[:, col:col + 1],
                    scalar=1.0 / HW, in1=half_sb, op0=OP.mult, op1=OP.add,
                )
            mv = [mv0, mv1]

            # group-level [mean_g, E[x^2]_g] via indicator matmul (values 1/8)
            gst_ps = ps_sm.tile([GROUPS, 2], f32, tag="small")
            nc.tensor.matmul(gst_ps, lhsT=aux_sb[:, 7:7 + GROUPS], rhs=mv[0],
                             start=True, stop=False)
            nc.tensor.matmul(gst_ps, lhsT=aux_sb[:, 7 + GROUPS:7 + 2 * GROUPS],
                             rhs=mv[1], start=False, stop=True)
            gst = const.tile([GROUPS, 2], f32)
            nc.vector.tensor_copy(gst, gst_ps)

            # var_g = E[x^2]_g - mean_g^2 + eps; rs = rsqrt(var) via the
            # bit-trick seed + 2 Newton iterations, all on the DVE
            varg = tmp.tile([GROUPS, 1], f32, tag="varg")
            nc.vector.tensor_mul(varg, gst[:, 0:1], gst[:, 0:1])
            nc.vector.tensor_sub(varg, gst[:, 1:2], varg)
            nc.vector.tensor_scalar_add(varg, varg, float(EPS))
            st = const.tile([GROUPS, 2], f32)  # [rs_g, -mu_g*rs_g]
            y = st[:, 0:1]
            i32 = mybir.dt.int32
            nc.vector.tensor_scalar(
                out=y.bitcast(i32), in0=varg.bitcast(i32),
                scalar1=1, scalar2=None, op0=OP.logical_shift_right,
            )
            nc.vector.tensor_scalar(
                out=y.bitcast(i32), in0=y.bitcast(i32),
                scalar1=-1, scalar2=0x5F3759DF, op0=OP.mult, op1=OP.add,
            )
            tnr = tmp.tile([GROUPS, 1], f32, tag="tnr")
            for it in range(3):
                nc.vector.tensor_mul(tnr, y, y)
                # tnr = (tnr * 0.5) * varg
                nc.vector.scalar_tensor_tensor(
                    out=tnr, in0=tnr, scalar=0.5, in1=varg,
                    op0=OP.mult, op1=OP.mult,
                )
                nc.vector.tensor_scalar(
                    out=tnr, in0=tnr, scalar1=-1.0, scalar2=1.5,
                    op0=OP.mult, op1=OP.add,
                )
                nc.vector.tensor_mul(y, y, tnr)
            nc.vector.tensor_mul(st[:, 1:2], gst[:, 0:1], st[:, 0:1])
            nc.vector.tensor_scalar_mul(st[:, 1:2], st[:, 1:2], -1.0)

            # per-channel [s_c, t_c]: gbc carries gn_weight, and W@gn_bias is
            # folded into the host-side raw biases, so the broadcast matmul
            # output is used directly
            sca = []
            for po in range(2):
                stc_ps = ps_sm.tile([P, 2], f32, tag="small")
                nc.tensor.matmul(stc_ps, lhsT=gbc_sb[:, po, :], rhs=st,
                                 start=True, stop=True)
                sc = const.tile([P, 2], f32, tag=f"sca{po}")
                nc.vector.tensor_copy(sc, stc_ps)
                sca.append(sc)

            # scaled fp16 qkv weights: W'[.,c] = W[.,c] * s_c
            wsc = const.tile([P, 2, 3 * HD], adt)
            for po in range(2):
                nc.vector.tensor_scalar_mul(wsc[:, po, :], wqkv_sb[:, po, :],
                                            sca[po][:, 0:1])

            # effective biases: b' = W @ t + b_raw.
            # q and k adj land on both partition halves via col-packed N=1
            # matmuls; v adj on rows 0:64 only.
            bqe = const.tile([P, 1], f32)
            bke = const.tile([P, 1], f32)
            bve = const.tile([P, 1], f32)
            for (wofs, dst, rawcol) in ((0, bqe, 0), (HD, bke, 1),
                                        (2 * HD, bve, 2)):
                bp = ps_sm.tile([P, 1], f32, tag="small", name="bp")
                halves = (0, 1) if rawcol < 2 else (0,)
                for half in halves:
                    hs = slice(half * HD, (half + 1) * HD)
                    nc.tensor.matmul(bp[hs, :],
                                     lhsT=wqkv_sb[:, 0, wofs:wofs + HD],
                                     rhs=sca[0][:, 1:2], start=True, stop=False)
                    nc.tensor.matmul(bp[hs, :],
                                     lhsT=wqkv_sb[:, 1, wofs:wofs + HD],
                                     rhs=sca[1][:, 1:2], start=False, stop=True)
                nc.vector.tensor_add(dst[:HD * len(halves), :],
                                     bp[:HD * len(halves), :],
                                     aux_sb[:HD * len(halves),
                                            rawcol:rawcol + 1])
            nc.sync.dma_start(bve_d, bve[0:HD, :])

            # ---- qkv on x16; q and k duplicated onto partitions 64..127
            # via col-packed matmuls (concurrent on the PE array) ----
            qq_sb = big.tile([P, HW], adt)
            kk_sb = big.tile([P, HW], adt)
            vt_sb = big.tile([P, NJC, HD + 1], adt)
            nc.vector.tensor_copy(
                vt_sb[:, :, HD:HD + 1],
                ones_sb[:, None, :].to_broadcast([P, NJC, 1]),
            )
            for n in range(HW // 512):
                ns = slice(n * 512, (n + 1) * 512)
                for (wofs, bsb, dst) in ((0, bqe, qq_sb), (HD, bke, kk_sb)):
                    qp = ps_sm.tile([P, 512], f32, tag="small", name="qp")
                    for half in range(2):
                        hs = slice(half * HD, (half + 1) * HD)
                        nc.tensor.matmul(qp[hs, :],
                                         lhsT=wsc[:, 0, wofs:wofs + HD],
                                         rhs=x16[:, 0, ns],
                                         start=True, stop=False)
                        nc.tensor.matmul(qp[hs, :],
                                         lhsT=wsc[:, 1, wofs:wofs + HD],
                                         rhs=x16[:, 1, ns],
                                         start=False, stop=True)
                    if n < 2:
                        # first query block's q/k land via the (still idle)
                        # ScalarE so the DVE isn't the gate for the first exp
                        nc.scalar.add(dst[:, ns], qp, bsb)
                    else:
                        nc.vector.tensor_scalar_add(dst[:, ns], qp, bsb)
            # v^T directly: [positions, head_dim], chunked by 128 positions
            for jc in range(NJC):
                js = slice(jc * P, (jc + 1) * P)
                vp = ps_sm.tile([P, HD], f32, tag="small", name="vp")
                nc.tensor.matmul(vp, lhsT=x16[:, 0, js],
                                 rhs=wsc[:, 0, 2 * HD:3 * HD],
                                 start=True, stop=False)
                nc.tensor.matmul(vp, lhsT=x16[:, 1, js],
                                 rhs=wsc[:, 1, 2 * HD:3 * HD],
                                 start=False, stop=True)
                nc.vector.tensor_copy(vt_sb[:, jc, 0:HD], vp)

            # ---- attention + proj, blocked over queries. The previous
            # block's epilogue is emitted after the first two exp's of the
            # next block so the ScalarE never stalls at block boundaries ----
            SC = float(1.0 / np.sqrt(HD))
            pend = []

            def epilogue_pieces():
                """Yield the previous block's epilogue in 5 pieces so its PE
                work spreads across the next block's jc iterations instead of
                gapping the exp stream."""
                if not pend:
                    return
                ib0, pv0 = pend.pop()
                ibs0 = ib0 * IB
                oh16 = ohp.tile([HD, IB], adt, tag="oh16", name="oh16")
                nc.vector.tensor_copy(oh16, pv0[0:HD, :])
                den_sb = ohp.tile([1, IB], f32, tag="den", name="den_sb")
                nc.vector.tensor_copy(den_sb, pv0[HD:HD + 1, :])
                nc.sync.dma_start(den_d[ib0:ib0 + 1, :], den_sb)
                yield
                for mt in range(2):
                    for n2 in range(IB // 512):
                        pp = ps_sm.tile([P, 512], f32, tag="small", name="pp")
                        nc.tensor.matmul(
                            pp,
                            lhsT=wp_sb[:, mt * P:(mt + 1) * P],
                            rhs=oh16[:, n2 * 512:(n2 + 1) * 512],
                            start=True, stop=True,
                        )
                        sg = ostage.tile([P, 512], f32, tag="ostage", name="sg")
                        nc.vector.tensor_copy(sg, pp)
                        nc.sync.dma_start(
                            out_d[:, mt, ibs0 + n2 * 512: ibs0 + (n2 + 1) * 512],
                            sg)
                        yield

            def emit_epilogue():
                for _ in epilogue_pieces():
                    pass

            for ib in range(NIB):
                ibs = ib * IB
                pts = {}
                # S + exp for the first two key-chunks before the previous
                # block's epilogue claims the PE
                for jc in range(2):
                    st_ps = ps_st.tile([P, IB], f32, tag="st", name="st_ps")
                    nc.tensor.matmul(
                        st_ps[:, 0:512],
                        lhsT=kk_sb[0:HD, jc * P:(jc + 1) * P],
                        rhs=qq_sb[0:HD, ibs: ibs + 512],
                        start=True, stop=True,
                    )
                    nc.tensor.matmul(
                        st_ps[:, 512:1024],
                        lhsT=kk_sb[HD:P, jc * P:(jc + 1) * P],
                        rhs=qq_sb[HD:P, ibs + 512: ibs + 1024],
                        start=True, stop=True,
                    )
                    pt = ptp.tile([P, IB], adt, tag="pt", name="pt")
                    nc.scalar.activation(out=pt, in_=st_ps, func=AF.Exp,
                                         scale=SC)
                    pts[jc] = pt
                pieces = epilogue_pieces()
                next(pieces, None)  # oh16/den copies (DVE) before pv realloc
                pv_ps = ps_pv.tile([HD + 1, IB], f32, tag="pv", name="pv_ps")
                for jc in range(NJC):
                    if 2 <= jc <= 6:
                        next(pieces, None)  # one proj piece per jc
                    if jc in pts:
                        pt = pts.pop(jc)
                    else:
                        st_ps = ps_st.tile([P, IB], f32, tag="st", name="st_ps")
                        nc.tensor.matmul(
                            st_ps[:, 0:512],
                            lhsT=kk_sb[0:HD, jc * P:(jc + 1) * P],
                            rhs=qq_sb[0:HD, ibs: ibs + 512],
                            start=True, stop=True,
                        )
                        nc.tensor.matmul(
                            st_ps[:, 512:1024],
                            lhsT=kk_sb[HD:P, jc * P:(jc + 1) * P],
                            rhs=qq_sb[HD:P, ibs + 512: ibs + 1024],
                            start=True, stop=True,
                        )
                        pt = ptp.tile([P, IB], adt, tag="pt", name="pt")
                        nc.scalar.activation(out=pt, in_=st_ps, func=AF.Exp,
                                             scale=SC)
                    for n2 in range(IB // 512):
                        nc.tensor.matmul(
                            pv_ps[:, n2 * 512:(n2 + 1) * 512],
                            lhsT=vt_sb[:, jc, :],
                            rhs=pt[:, n2 * 512:(n2 + 1) * 512],
                            start=(jc == 0), stop=(jc == NJC - 1),
                        )
                pend.append((ib, pv_ps))
            emit_epilogue()
    nc.compile()
    return nc


def get_module(mm=MM_MODE):
    if mm not in _module_cache:
        _module_cache[mm] = _build_module(mm)
    return _module_cache[mm]


def _group_mats(gn_weight):
    gmat = np.zeros((P, 2, GROUPS), np.float32)
    gbc = np.zeros((GROUPS, 2, P), np.float32)
    for po in range(2):
        for pi in range(P):
            c = po * P + pi
            g = c // 8
            gmat[pi, po, g] = 1.0 / 8.0
            gbc[g, po, pi] = gn_weight[c]
    return gmat, gbc


def make_in_maps(x, gn_weight, gn_bias, qkv_weight, qkv_bias,
                 proj_weight=None, mm=None):
    mm = mm or MM_MODE
    wp_np = np.float16 if mm == "f16" else np.float32
    x = np.asarray(x, np.float32)
    gn_weight = np.asarray(gn_weight, np.float32)
    gn_bias = np.asarray(gn_bias, np.float32)
    qkv_weight = np.asarray(qkv_weight, np.float32)
    qkv_bias = np.asarray(qkv_bias, np.float32)
    gmat, gbc = _group_mats(gn_weight)
    gnw = np.ascontiguousarray(gn_weight.reshape(2, P).T)   # [128, 2]
    gnb = np.ascontiguousarray(gn_bias.reshape(2, P).T)

    def wslice(row0):
        w = qkv_weight[row0:row0 + HD, :]            # [64, 256]
        return w.T.reshape(2, P, HD).transpose(1, 0, 2)

    def bias2(off):
        # raw bias + W @ gn_bias (the additive part of the GN affine)
        b = (qkv_bias[off:off + HD]
             + qkv_weight[off:off + HD, :] @ gn_bias).reshape(HD, 1)
        return np.vstack([b, b])

    wps = [None] * NH
    if proj_weight is not None:
        pw = np.asarray(proj_weight, np.float32)
        wps = [np.ascontiguousarray(
            pw[:, h * HD:(h + 1) * HD].T.astype(wp_np)) for h in range(NH)]

    in_maps = []
    for b in range(B):
        xt = np.ascontiguousarray(
            x[b].reshape(2, P, HW).transpose(1, 0, 2).astype(wp_np))
        for h in range(NH):
            wqkv = np.concatenate(
                [wslice(h * HD), wslice(C + h * HD), wslice(2 * C + h * HD)],
                axis=2).astype(np.float32)
            bv = np.zeros((P, 1), np.float32)
            vrow = 2 * C + h * HD
            bv[0:HD, 0] = (qkv_bias[vrow:vrow + HD]
                           + qkv_weight[vrow:vrow + HD, :] @ gn_bias)
            aux = np.concatenate(
                [bias2(h * HD), bias2(C + h * HD), bv,
                 gnw[:, 0:1], gnw[:, 1:2], gnb[:, 0:1], gnb[:, 1:2],
                 gmat[:, 0, :], gmat[:, 1, :]], axis=1).astype(np.float32)
            in_maps.append({
                "x": xt,
                "wqkv": np.ascontiguousarray(wqkv),
                "wp": wps[h],
                "aux": np.ascontiguousarray(aux),
                "gbc": gbc,
            })
    return in_maps


def combine_outputs(results, x, proj_weight, proj_bias):
    """results: 8 dicts with 'out' [128,2,HW], 'den' [NIB,IB], 'bve' [HD,1]."""
    x = np.asarray(x, np.float32)
    proj_weight = np.asarray(proj_weight, np.float32)
    proj_bias = np.asarray(proj_bias, np.float32)
    y = np.empty((B, C, H, W), np.float32)
    for b in range(B):
        acc = x[b].reshape(C, HW) + proj_bias[:, None]
        for h in range(NH):
            r = results[b * NH + h]
            part = np.asarray(r["out"]).transpose(1, 0, 2).reshape(C, HW)
            den = np.asarray(r["den"]).reshape(HW)
            bve = np.asarray(r["bve"]).reshape(HD)
            ch = proj_weight[:, h * HD:(h + 1) * HD] @ bve
            acc = acc + part / den[None, :] + ch[:, None]
        y[b] = acc.reshape(C, H, W)
    return y


def kernel(x, gn_weight, gn_bias, qkv_weight, qkv_bias, proj_weight, proj_bias):
    from concourse.bass_utils import run_bass_kernel_spmd

    nc = get_module()
    in_maps = make_in_maps(x, gn_weight, gn_bias, qkv_weight, qkv_bias,
                           proj_weight=proj_weight)
    res = run_bass_kernel_spmd(nc, in_maps, core_ids=list(range(NCORES)))
    return combine_outputs(res.results, x, proj_weight, proj_bias)


# revision 41
# speedup vs baseline: 1.0149x; 1.0149x over previous
"""Trainium2 Bass kernel for nn_Attention2d.

Computation: GroupNorm(32 groups) -> 1x1 qkv conv -> 4-head attention over
H*W=4096 positions -> 1x1 proj conv -> residual add.

Sharding: one (batch, head) pair per NeuronCore (B=2 x NH=4 = 8 cores).
Each core:
  - GroupNorm stats of its batch slice; the affine normalization is folded
    into the qkv weights (W' = W*s per input channel) and effective biases
    (b' = W@t + b), so the x-sized tensor is only cast to fp16 once
  - its head's q/k (with effective bias) and v^T (bias exported to host)
  - S^T = k^T q in [keys-on-partitions, queries-on-free] layout
    (no max-subtraction: |S/8| <~ 6 so exp is safe in fp32)
  - P^T = exp(S^T/8); PV via matmul with lhsT = [v^T | ones]  -> the ones
    column yields the softmax denominators for free (row 64 of the output)
  - proj partial = Wp[:, head]^T @ PV_raw  (un-normalized)
Host: out[b] = x[b] + proj_bias + sum_h (partial_h/denom_h + Wp_h @ bve_h)
(the softmax normalization and the constant v-bias commute through proj).

PE-array packing: the S matmuls contract over only hd=64 partitions, so q and
k are duplicated onto partitions 64..127 (by col-packed qkv matmuls that cost
no extra PE time) and each S^T tile is computed as two concurrent matmuls on
row-groups (0,0) and (64,0).

Matmul dtypes: qkv/attention/proj matmuls use float16 operands (1 cycle/row,
fast weight loads); GroupNorm matmuls, softmax denominators and all
reductions stay fp32. GroupNorm's rsqrt runs on the DVE (bit-trick seed +
Newton) so the ScalarE keeps a single Exp table set for the whole kernel.
"""

import numpy as np

B, C, H, W = 2, 256, 64, 64
HW = H * W           # 4096
GROUPS = 32
NH = 4
HD = C // NH         # 64
EPS = 1e-5
P = 128
IB = 1024            # query block (PSUM-sized)
NIB = HW // IB       # 4
NJC = HW // P        # 32 key chunks
NCORES = B * NH

# "f32": exact fp32 everywhere (slow). "f32r": float32r operands
# (TF32-like rounding, ~3 cycles/row on HW). "f16": float16 operands.
MM_MODE = "f16"

# aux column layout: 0 bq2, 1 bk2, 2 bv (rows 0:64), 3+po gnw, 5+po gnb,
# 7+32*po gmat
NAUX = 7 + 2 * GROUPS

_module_cache = {}


def _build_module(mm=MM_MODE):
    import concourse.bacc as bacc
    import concourse.tile as tile
    import concourse.mybir as mybir

    dt = mybir.dt
    f32 = dt.float32
    AF = mybir.ActivationFunctionType
    OP = mybir.AluOpType
    if mm == "f32":
        adt = f32
    elif mm == "f32r":
        adt = dt.float32r
    elif mm == "f16":
        adt = dt.float16
    else:
        raise ValueError(mm)

    nc = bacc.Bacc(trn_type="TRN2", target_bir_lowering=False, debug=False)

    # ---- DRAM I/O (per-core tensors; host prepares layouts) ----
    # channel layout everywhere: c = po*128 + pi  ->  [pi, po, ...]
    # x arrives already cast to the attention dtype (host-side cast); the
    # GroupNorm statistics absorb the rounding (it averages out over 32k
    # elements per group).
    x_d = nc.dram_tensor("x", [P, 2, HW], adt, kind="ExternalInput").ap()
    # packed raw qkv weight slices: [wq | wk | wv] along the last axis, fp32
    wqkv_d = nc.dram_tensor("wqkv", [P, 2, 3 * HD], f32, kind="ExternalInput").ap()
    wp_d = nc.dram_tensor("wp", [HD, C], adt if mm == "f16" else f32,
                          kind="ExternalInput").ap()
    aux_d = nc.dram_tensor("aux", [P, NAUX], f32, kind="ExternalInput").ap()
    gbc_d = nc.dram_tensor("gbc", [GROUPS, 2, P], f32, kind="ExternalInput").ap()
    out_d = nc.dram_tensor("out", [P, 2, HW], f32, kind="ExternalOutput").ap()
    den_d = nc.dram_tensor("den", [NIB, IB], f32, kind="ExternalOutput").ap()
    bve_d = nc.dram_tensor("bve", [HD, 1], f32, kind="ExternalOutput").ap()

    with tile.TileContext(nc) as tc:
        with (
            tc.tile_pool(name="const", bufs=1) as const,
            tc.tile_pool(name="big", bufs=1) as big,
            tc.tile_pool(name="tmp", bufs=3) as tmp,
            tc.tile_pool(name="pt", bufs=4) as ptp,
            tc.tile_pool(name="oh", bufs=2) as ohp,
            tc.tile_pool(name="ostage", bufs=3) as ostage,
            tc.tile_pool(name="ps_st", bufs=2, space="PSUM") as ps_st,
            tc.tile_pool(name="ps_pv", bufs=1, space="PSUM") as ps_pv,
            tc.tile_pool(name="ps_sm", bufs=2, space="PSUM") as ps_sm,
        ):
            eps_sb = const.tile([GROUPS, 1], f32)
            nc.vector.memset(eps_sb, EPS)
            ones_sb = const.tile([P, 1], f32)
            nc.vector.memset(ones_sb, 1.0)
            # Touch Exp immediately so walrus's single ACT table load runs
            # during the DMA-in phase.
            warm_sb = tmp.tile([GROUPS, 1], f32, tag="warm")
            nc.scalar.activation(out=warm_sb, in_=eps_sb, func=AF.Exp, scale=1.0)

            # ---- x first (chunked so stats start early); per-channel
            # moment accumulation split across DVE (po=0, bn_stats) and
            # ScalarE (po=1, Identity/Square with accum_out) ----
            x16 = big.tile([P, 2, HW], adt)
            stats0 = tmp.tile([P, 8, 6], f32, tag="bnstats0", name="stats0")
            stats1 = tmp.tile([P, 4, 6], f32, tag="bnstats1", name="stats1")
            psums1 = tmp.tile([P, 2], f32, tag="psums1", name="psums1")
            scratch = tmp.tile([P, HW // 2], adt, tag="scratch", name="scratch")
            HB = HW // 2
            for c in range(2):
                cs = slice(c * HB, (c + 1) * HB)
                # po=1 via the ScalarE HWDGE queue (parallel issue with po=0)
                nc.scalar.dma_start(x16[:, 1, cs], x_d[:, 1, cs])
                nc.sync.dma_start(x16[:, 0, cs], x_d[:, 0, cs])
                if c == 0:
                    # first po=1 half: two ScalarE accumulation passes
                    nc.scalar.activation(out=scratch, in_=x16[:, 1, cs],
                                         func=AF.Identity,
                                         accum_out=psums1[:, 0:1])
                    nc.scalar.activation(out=scratch, in_=x16[:, 1, cs],
                                         func=AF.Square,
                                         accum_out=psums1[:, 1:2])
                else:
                    # second po=1 half: DVE bn_stats (ScalarE is the long pole)
                    for s in range(4):
                        nc.vector.bn_stats(
                            out=stats1[:, s, :],
                            in_=x16[:, 1, HB + s * 512:HB + (s + 1) * 512],
                        )
                for s in range(4 * c, 4 * c + 4):
                    nc.vector.bn_stats(
                        out=stats0[:, s, :],
                        in_=x16[:, 0, s * 512:(s + 1) * 512],
                    )
            # weights ride the ScalarE HWDGE queue so the Sync queue stays
            # dedicated to x
            wqkv_sb = const.tile([P, 2, 3 * HD], f32)
            nc.scalar.dma_start(wqkv_sb, wqkv_d)
            aux_sb = const.tile([P, NAUX], f32)
            nc.scalar.dma_start(aux_sb, aux_d)
            gbc_sb = const.tile([GROUPS, 2, P], f32)
            nc.scalar.dma_start(gbc_sb, gbc_d)
            wp_sb = const.tile([HD, C], adt if mm == "f16" else f32)
            nc.scalar.dma_start(wp_sb, wp_d)

            # per-channel [mean_c, E[x^2]_c] for both po halves.
            # po=0: straight bn_aggr. po=1: combine ScalarE raw sums (first
            # half) with bn stats (second half):
            #   mean = s0/4096 + mean_c1/2;  Ex2 = q0/4096 + (var+mean^2)_c1/2
            mv0 = const.tile([P, 2], f32, tag="mv0")
            nc.vector.bn_aggr(out=mv0, in_=stats0)
            msq = tmp.tile([P, 1], f32, tag="msq")
            nc.vector.tensor_mul(msq, mv0[:, 0:1], mv0[:, 0:1])
            nc.vector.tensor_add(mv0[:, 1:2], mv0[:, 1:2], msq)
            mvq = tmp.tile([P, 2], f32, tag="mvq", name="mvq")
            nc.vector.bn_aggr(out=mvq, in_=stats1)
            msq1 = tmp.tile([P, 1], f32, tag="msq1", name="msq1")
            nc.vector.tensor_mul(msq1, mvq[:, 0:1], mvq[:, 0:1])
            nc.vector.tensor_add(mvq[:, 1:2], mvq[:, 1:2], msq1)  # Ex2_c1
            mv1 = const.tile([P, 2], f32, tag="mv1")
            for col in range(2):
                half_sb = tmp.tile([P, 1], f32, tag="halfsb", name="half_sb")
                nc.vector.tensor_scalar_mul(half_sb, mvq[:, col:col + 1], 0.5)
                nc.vector.scalar_tensor_tensor(
                    out=mv1[:, col:col + 1], in0=psums1[:, col:col + 1],
                    scalar=1.0 / HW, in1=half_sb, op0=OP.mult, op1=OP.add,
                )
            mv = [mv0, mv1]

            # group-level [mean_g, E[x^2]_g] via indicator matmul (values 1/8)
            gst_ps = ps_sm.tile([GROUPS, 2], f32, tag="small")
            nc.tensor.matmul(gst_ps, lhsT=aux_sb[:, 7:7 + GROUPS], rhs=mv[0],
                             start=True, stop=False)
            nc.tensor.matmul(gst_ps, lhsT=aux_sb[:, 7 + GROUPS:7 + 2 * GROUPS],
                             rhs=mv[1], start=False, stop=True)
            gst = const.tile([GROUPS, 2], f32)
            nc.vector.tensor_copy(gst, gst_ps)

            # var_g = E[x^2]_g - mean_g^2 + eps; rs = rsqrt(var) via the
            # bit-trick seed + 2 Newton iterations, all on the DVE
            varg = tmp.tile([GROUPS, 1], f32, tag="varg")
            nc.vector.tensor_mul(varg, gst[:, 0:1], gst[:, 0:1])
            nc.vector.tensor_sub(varg, gst[:, 1:2], varg)
            nc.vector.tensor_scalar_add(varg, varg, float(EPS))
            st = const.tile([GROUPS, 2], f32)  # [rs_g, -mu_g*rs_g]
            y = st[:, 0:1]
            i32 = mybir.dt.int32
            nc.vector.tensor_scalar(
                out=y.bitcast(i32), in0=varg.bitcast(i32),
                scalar1=1, scalar2=None, op0=OP.logical_shift_right,
            )
            nc.vector.tensor_scalar(
                out=y.bitcast(i32), in0=y.bitcast(i32),
                scalar1=-1, scalar2=0x5F3759DF, op0=OP.mult, op1=OP.add,
            )
            tnr = tmp.tile([GROUPS, 1], f32, tag="tnr")
            for it in range(3):
                nc.vector.tensor_mul(tnr, y, y)
                # tnr = (tnr * 0.5) * varg
                nc.vector.scalar_tensor_tensor(
                    out=tnr, in0=tnr, scalar=0.5, in1=varg,
                    op0=OP.mult, op1=OP.mult,
                )
                nc.vector.tensor_scalar(
                    out=tnr, in0=tnr, scalar1=-1.0, scalar2=1.5,
                    op0=OP.mult, op1=OP.add,
                )
                nc.vector.tensor_mul(y, y, tnr)
            nc.vector.tensor_mul(st[:, 1:2], gst[:, 0:1], st[:, 0:1])
            nc.vector.tensor_scalar_mul(st[:, 1:2], st[:, 1:2], -1.0)

            # per-channel [s_c, t_c]: gbc carries gn_weight, and W@gn_bias is
            # folded into the host-side raw biases, so the broadcast matmul
            # output is used directly
            sca = []
            for po in range(2):
                stc_ps = ps_sm.tile([P, 2], f32, tag="small")
                nc.tensor.matmul(stc_ps, lhsT=gbc_sb[:, po, :], rhs=st,
                                 start=True, stop=True)
                sc = const.tile([P, 2], f32, tag=f"sca{po}")
                nc.vector.tensor_copy(sc, stc_ps)
                sca.append(sc)

            # scaled fp16 qkv weights: W'[.,c] = W[.,c] * s_c
            wsc = const.tile([P, 2, 3 * HD], adt)
            for po in range(2):
                nc.vector.tensor_scalar_mul(wsc[:, po, :], wqkv_sb[:, po, :],
                                            sca[po][:, 0:1])

            # effective biases: b' = W @ t + b_raw.
            # q and k adj land on both partition halves via col-packed N=1
            # matmuls; v adj on rows 0:64 only.
            bqe = const.tile([P, 1], f32)
            bke = const.tile([P, 1], f32)
            bve = const.tile([P, 1], f32)
            for (wofs, dst, rawcol) in ((0, bqe, 0), (HD, bke, 1),
                                        (2 * HD, bve, 2)):
                bp = ps_sm.tile([P, 1], f32, tag="small", name="bp")
                halves = (0, 1) if rawcol < 2 else (0,)
                for half in halves:
                    hs = slice(half * HD, (half + 1) * HD)
                    nc.tensor.matmul(bp[hs, :],
                                     lhsT=wqkv_sb[:, 0, wofs:wofs + HD],
                                     rhs=sca[0][:, 1:2], start=True, stop=False)
                    nc.tensor.matmul(bp[hs, :],
                                     lhsT=wqkv_sb[:, 1, wofs:wofs + HD],
                                     rhs=sca[1][:, 1:2], start=False, stop=True)
                nc.vector.tensor_add(dst[:HD * len(halves), :],
                                     bp[:HD * len(halves), :],
                                     aux_sb[:HD * len(halves),
                                            rawcol:rawcol + 1])
            nc.sync.dma_start(bve_d, bve[0:HD, :])

            # ---- qkv on x16; q and k duplicated onto partitions 64..127
            # via col-packed matmuls (concurrent on the PE array) ----
            qq_sb = big.tile([P, HW], adt)
            kk_sb = big.tile([P, HW], adt)
            vt_sb = big.tile([P, NJC, HD + 1], adt)
            nc.vector.tensor_copy(
                vt_sb[:, :, HD:HD + 1],
                ones_sb[:, None, :].to_broadcast([P, NJC, 1]),
            )
            def emit_qk_chunk(n, kind):
                ns = slice(n * 512, (n + 1) * 512)
                wofs, bsb, dst = ((0, bqe, qq_sb) if kind == "q"
                                  else (HD, bke, kk_sb))
                qp = ps_sm.tile([P, 512], f32, tag="small", name="qp")
                for half in range(2):
                    hs = slice(half * HD, (half + 1) * HD)
                    nc.tensor.matmul(qp[hs, :],
                                     lhsT=wsc[:, 0, wofs:wofs + HD],
                                     rhs=x16[:, 0, ns],
                                     start=True, stop=False)
                    nc.tensor.matmul(qp[hs, :],
                                     lhsT=wsc[:, 1, wofs:wofs + HD],
                                     rhs=x16[:, 1, ns],
                                     start=False, stop=True)
                if n < 2:
                    # first query block's q/k land via the (still idle)
                    # ScalarE so the DVE isn't the gate for the first exp
                    nc.scalar.add(dst[:, ns], qp, bsb)
                else:
                    nc.vector.tensor_scalar_add(dst[:, ns], qp, bsb)

            def emit_vt_chunk(jc):
                js = slice(jc * P, (jc + 1) * P)
                vp = ps_sm.tile([P, HD], f32, tag="small", name="vp")
                nc.tensor.matmul(vp, lhsT=x16[:, 0, js],
                                 rhs=wsc[:, 0, 2 * HD:3 * HD],
                                 start=True, stop=False)
                nc.tensor.matmul(vp, lhsT=x16[:, 1, js],
                                 rhs=wsc[:, 1, 2 * HD:3 * HD],
                                 start=False, stop=True)
                nc.vector.tensor_copy(vt_sb[:, jc, 0:HD], vp)

            # Emit only what block 0 needs up front: q/k for its 1024
            # queries, all k chunks (S sweeps every key), and the first 8
            # v^T chunks. The rest is deferred into block 0's jc loop so the
            # first exps aren't starved of PE time.
            for n in range(2):
                emit_qk_chunk(n, "q")
                emit_qk_chunk(n, "k")
            for n in range(2, HW // 512):
                emit_qk_chunk(n, "k")
            for jc in range(8):
                emit_vt_chunk(jc)
            deferred = ([("vt", jc) for jc in range(8, NJC)]
                        + [("q", n) for n in range(2, HW // 512)])

            # ---- attention + proj, blocked over queries. The previous
            # block's epilogue is emitted after the first two exp's of the
            # next block so the ScalarE never stalls at block boundaries ----
            SC = float(1.0 / np.sqrt(HD))
            pend = []

            def epilogue_pieces():
                """Yield the previous block's epilogue in 5 pieces so its PE
                work spreads across the next block's jc iterations instead of
                gapping the exp stream."""
                if not pend:
                    return
                ib0, pv0 = pend.pop()
                ibs0 = ib0 * IB
                oh16 = ohp.tile([HD, IB], adt, tag="oh16", name="oh16")
                nc.vector.tensor_copy(oh16, pv0[0:HD, :])
                den_sb = ohp.tile([1, IB], f32, tag="den", name="den_sb")
                nc.vector.tensor_copy(den_sb, pv0[HD:HD + 1, :])
                nc.sync.dma_start(den_d[ib0:ib0 + 1, :], den_sb)
                yield
                for mt in range(2):
                    for n2 in range(IB // 512):
                        pp = ps_sm.tile([P, 512], f32, tag="small", name="pp")
                        nc.tensor.matmul(
                            pp,
                            lhsT=wp_sb[:, mt * P:(mt + 1) * P],
                            rhs=oh16[:, n2 * 512:(n2 + 1) * 512],
                            start=True, stop=True,
                        )
                        sg = ostage.tile([P, 512], f32, tag="ostage", name="sg")
                        nc.vector.tensor_copy(sg, pp)
                        nc.sync.dma_start(
                            out_d[:, mt, ibs0 + n2 * 512: ibs0 + (n2 + 1) * 512],
                            sg)
                        yield

            def emit_epilogue():
                for _ in epilogue_pieces():
                    pass

            for ib in range(NIB):
                ibs = ib * IB
                pts = {}
                # S + exp for the first two key-chunks before the previous
                # block's epilogue claims the PE
                for jc in range(2):
                    st_ps = ps_st.tile([P, IB], f32, tag="st", name="st_ps")
                    nc.tensor.matmul(
                        st_ps[:, 0:512],
                        lhsT=kk_sb[0:HD, jc * P:(jc + 1) * P],
                        rhs=qq_sb[0:HD, ibs: ibs + 512],
                        start=True, stop=True,
                    )
                    nc.tensor.matmul(
                        st_ps[:, 512:1024],
                        lhsT=kk_sb[HD:P, jc * P:(jc + 1) * P],
                        rhs=qq_sb[HD:P, ibs + 512: ibs + 1024],
                        start=True, stop=True,
                    )
                    pt = ptp.tile([P, IB], adt, tag="pt", name="pt")
                    nc.scalar.activation(out=pt, in_=st_ps, func=AF.Exp,
                                         scale=SC)
                    pts[jc] = pt
                pieces = epilogue_pieces()
                next(pieces, None)  # oh16/den copies (DVE) before pv realloc
                pv_ps = ps_pv.tile([HD + 1, IB], f32, tag="pv", name="pv_ps")
                for jc in range(NJC):
                    if jc >= 2 and deferred:
                        kind, arg = deferred.pop(0)
                        if kind == "vt":
                            emit_vt_chunk(arg)
                        else:
                            emit_qk_chunk(arg, "q")
                    if 2 <= jc <= 6:
                        next(pieces, None)  # one proj piece per jc
                    if jc in pts:
                        pt = pts.pop(jc)
                    else:
                        st_ps = ps_st.tile([P, IB], f32, tag="st", name="st_ps")
                        nc.tensor.matmul(
                            st_ps[:, 0:512],
                            lhsT=kk_sb[0:HD, jc * P:(jc + 1) * P],
                            rhs=qq_sb[0:HD, ibs: ibs + 512],
                            start=True, stop=True,
                        )
                        nc.tensor.matmul(
                            st_ps[:, 512:1024],
                            lhsT=kk_sb[HD:P, jc * P:(jc + 1) * P],
                            rhs=qq_sb[HD:P, ibs + 512: ibs + 1024],
                            start=True, stop=True,
                        )
                        pt = ptp.tile([P, IB], adt, tag="pt", name="pt")
                        nc.scalar.activation(out=pt, in_=st_ps, func=AF.Exp,
                                             scale=SC)
                    for n2 in range(IB // 512):
                        nc.tensor.matmul(
                            pv_ps[:, n2 * 512:(n2 + 1) * 512],
                            lhsT=vt_sb[:, jc, :],
                            rhs=pt[:, n2 * 512:(n2 + 1) * 512],
                            start=(jc == 0), stop=(jc == NJC - 1),
                        )
                pend.append((ib, pv_ps))
            emit_epilogue()
    nc.compile()
    return nc


def get_module(mm=MM_MODE):
    if mm not in _module_cache:
        _module_cache[mm] = _build_module(mm)
    return _module_cache[mm]


def _group_mats(gn_weight):
    gmat = np.zeros((P, 2, GROUPS), np.float32)
    gbc = np.zeros((GROUPS, 2, P), np.float32)
    for po in range(2):
        for pi in range(P):
            c = po * P + pi
            g = c // 8
            gmat[pi, po, g] = 1.0 / 8.0
            gbc[g, po, pi] = gn_weight[c]
    return gmat, gbc


def make_in_maps(x, gn_weight, gn_bias, qkv_weight, qkv_bias,
                 proj_weight=None, mm=None):
    mm = mm or MM_MODE
    wp_np = np.float16 if mm == "f16" else np.float32
    x = np.asarray(x, np.float32)
    gn_weight = np.asarray(gn_weight, np.float32)
    gn_bias = np.asarray(gn_bias, np.float32)
    qkv_weight = np.asarray(qkv_weight, np.float32)
    qkv_bias = np.asarray(qkv_bias, np.float32)
    gmat, gbc = _group_mats(gn_weight)
    gnw = np.ascontiguousarray(gn_weight.reshape(2, P).T)   # [128, 2]
    gnb = np.ascontiguousarray(gn_bias.reshape(2, P).T)

    def wslice(row0):
        w = qkv_weight[row0:row0 + HD, :]            # [64, 256]
        return w.T.reshape(2, P, HD).transpose(1, 0, 2)

    def bias2(off):
        # raw bias + W @ gn_bias (the additive part of the GN affine)
        b = (qkv_bias[off:off + HD]
             + qkv_weight[off:off + HD, :] @ gn_bias).reshape(HD, 1)
        return np.vstack([b, b])

    wps = [None] * NH
    if proj_weight is not None:
        pw = np.asarray(proj_weight, np.float32)
        wps = [np.ascontiguousarray(
            pw[:, h * HD:(h + 1) * HD].T.astype(wp_np)) for h in range(NH)]

    in_maps = []
    for b in range(B):
        xt = np.ascontiguousarray(
            x[b].reshape(2, P, HW).transpose(1, 0, 2).astype(wp_np))
        for h in range(NH):
            wqkv = np.concatenate(
                [wslice(h * HD), wslice(C + h * HD), wslice(2 * C + h * HD)],
                axis=2).astype(np.float32)
            bv = np.zeros((P, 1), np.float32)
            vrow = 2 * C + h * HD
            bv[0:HD, 0] = (qkv_bias[vrow:vrow + HD]
                           + qkv_weight[vrow:vrow + HD, :] @ gn_bias)
            aux = np.concatenate(
                [bias2(h * HD), bias2(C + h * HD), bv,
                 gnw[:, 0:1], gnw[:, 1:2], gnb[:, 0:1], gnb[:, 1:2],
                 gmat[:, 0, :], gmat[:, 1, :]], axis=1).astype(np.float32)
            in_maps.append({
                "x": xt,
                "wqkv": np.ascontiguousarray(wqkv),
                "wp": wps[h],
                "aux": np.ascontiguousarray(aux),
                "gbc": gbc,
            })
    return in_maps


def combine_outputs(results, x, proj_weight, proj_bias):
    """results: 8 dicts with 'out' [128,2,HW], 'den' [NIB,IB], 'bve' [HD,1]."""
    x = np.asarray(x, np.float32)
    proj_weight = np.asarray(proj_weight, np.float32)
    proj_bias = np.asarray(proj_bias, np.float32)
    y = np.empty((B, C, H, W), np.float32)
    for b in range(B):
        acc = x[b].reshape(C, HW) + proj_bias[:, None]
        for h in range(NH):
            r = results[b * NH + h]
            part = np.asarray(r["out"]).transpose(1, 0, 2).reshape(C, HW)
            den = np.asarray(r["den"]).reshape(HW)
            bve = np.asarray(r["bve"]).reshape(HD)
            ch = proj_weight[:, h * HD:(h + 1) * HD] @ bve
            acc = acc + part / den[None, :] + ch[:, None]
        y[b] = acc.reshape(C, H, W)
    return y


def kernel(x, gn_weight, gn_bias, qkv_weight, qkv_bias, proj_weight, proj_bias):
    from concourse.bass_utils import run_bass_kernel_spmd

    nc = get_module()
    in_maps = make_in_maps(x, gn_weight, gn_bias, qkv_weight, qkv_bias,
                           proj_weight=proj_weight)
    res = run_bass_kernel_spmd(nc, in_maps, core_ids=list(range(NCORES)))
    return combine_outputs(res.results, x, proj_weight, proj_bias)
